# revision 1
# baseline (speedup 1.0000x reference)
"""Trainium2 Bass kernel for nn_Bottleneck_11416023073044 (RFAConv bottleneck).

Sharding: pure data parallelism — 1 batch sample per NeuronCore (8 cores).

Per-core pipeline (channel-major layouts, [partition, spatial] tiles):
  cv1:    h = silu(a1*(W1 @ x) + c1)            fp32r matmuls, ACT silu
          written into a zero-padded 82x82 bf16 frame `hp`
  strips: for each channel group g (14 ch x 9 patch-idx = 126 partitions,
          n-major interleave: partition = i*ncg + cl), SBUF->SBUF DMAs build
          shifted patch strips hp9[(i,cl), y, x] = hp[c, y+dy, x+dx]
  z:      block-pattern matmul  z = zb_g^T @ hp9        (bf16, PSUM fp32)
  e:      e9 = exp(z + cg9)                              ACT -> bf16
  D:      D  = ones_c^T @ e9   -> [ncg, S] PSUM          (sum over patch idx)
  lnD:    lnD = Ln(D)                                    ACT -> fp32
  -lnD:   psum_z += (-rep)^T @ lnD                       fp32r accumulate
  w:      w9 = exp(z + cg9 - lnD)                        ACT -> bf16 (softmax)
  q:      q = hp9 * w9                                   DVE bf16 2x
  out:    psum_o[half] += wc9_g^T @ q  over groups       bf16 matmuls
  final:  out = x + relu(a2*psum_o + c2)                 ACT + DVE, DMA out
"""
import numpy as np
import ml_dtypes

EPS = 1e-5
B, C1, C2, H, W = 8, 256, 256, 80, 80
C_ = C2 // 2          # 128
NG = 10               # channel groups
GC = 14               # channels per group (last group has 2)
HP = H + 2            # 82
S = H * W             # 6400
CH_ROWS = 5           # rows per compute chunk
CS = CH_ROWS * W      # 400 columns per chunk
NCH = H // CH_ROWS    # 16 chunks
SC_ROWS = 40          # rows per strip super-chunk
NSC = H // SC_ROWS    # 2 super-chunks


def _grp(g):
    c0 = g * GC
    ncg = min(GC, C_ - c0)
    return c0, ncg, 9 * ncg


def _fold_constants(W1, g1, b1, m1, v1, Wg, bg, gg, bgw, mg, vg, Wc, bc, g2, b2,
                    m2, v2):
    """Fold BN affines and build the interleaved-layout stationaries.

    Interleave (n-major): partition k = i*ncg + cl for patch index i,
    channel-in-group cl.  Same for the output index m = n*ncg + cl.
    """
    f32 = np.float32
    bf16 = ml_dtypes.bfloat16
    cst = {}
    a1 = (g1 / np.sqrt(v1 + EPS)).astype(f32)
    c1 = (b1 - m1 * a1).astype(f32)
    cst['a1c1'] = np.stack([a1, c1], axis=1)                  # [128, 2] f32

    cst['w1t'] = np.ascontiguousarray(W1.T).astype(f32)       # [256, 128] f32

    ag = gg / np.sqrt(vg + EPS)                               # [128, 9]
    A = (ag[:, :, None] * Wg).astype(f32)                     # [c, n, i]
    cg = (ag * (bg - mg) + bgw).astype(f32)                   # [128, 9]

    zb = np.zeros((126, NG, 126), f32)
    ones_c = np.zeros((126, 2, GC), f32)    # variant 0: ncg=14, 1: ncg=2
    negrep = np.zeros((GC, 2, 126), f32)
    cg9 = np.zeros((126, NG), f32)
    wc9 = np.zeros((126, NG, C2), f32)
    for g in range(NG):
        c0, ncg, P = _grp(g)
        v = 0 if ncg == GC else 1
        for cl in range(ncg):
            c = c0 + cl
            for n in range(9):
                m = n * ncg + cl
                cg9[m, g] = cg[c, n]
                wc9[m, g, :] = Wc[:, c, n]
                for i in range(9):
                    zb[i * ncg + cl, g, m] = A[c, n, i]
        if g in (0, NG - 1):
            for cl in range(ncg):
                for i in range(9):
                    ones_c[i * ncg + cl, v, cl] = 1.0
                for n in range(9):
                    negrep[cl, v, n * ncg + cl] = -1.0
    cst['zb'] = zb.astype(bf16)
    cst['ones_c'] = ones_c.astype(bf16)
    cst['negrep'] = negrep.astype(f32)
    cst['cg9'] = cg9
    cst['wc9'] = wc9.astype(bf16)

    a2 = (g2 / np.sqrt(v2 + EPS)).astype(f32)
    c2 = (b2 + a2 * (bc - m2)).astype(f32)
    a2c2 = np.zeros((C_, 2, 2), f32)
    for h in range(2):
        a2c2[:, h, 0] = a2[h * C_:(h + 1) * C_]
        a2c2[:, h, 1] = c2[h * C_:(h + 1) * C_]
    cst['a2c2'] = a2c2
    return cst


_PROGRAM = None


def _build_program():
    import concourse.bass as bass
    import concourse.tile as tile
    from concourse import mybir

    dt = mybir.dt
    AF = mybir.ActivationFunctionType

    nc = bass.Bass("TRN2", target_bir_lowering=False, debug=False)

    xs_d = nc.dram_tensor("xs", [C1, S], dt.float32r, kind="ExternalInput")
    w1t_d = nc.dram_tensor("w1t", [C1, C_], dt.float32r, kind="ExternalInput")
    a1c1_d = nc.dram_tensor("a1c1", [C_, 2], dt.float32, kind="ExternalInput")
    zb_d = nc.dram_tensor("zb", [126, NG, 126], dt.bfloat16, kind="ExternalInput")
    ones_d = nc.dram_tensor("ones_c", [126, 2, GC], dt.bfloat16, kind="ExternalInput")
    negrep_d = nc.dram_tensor("negrep", [GC, 2, 126], dt.float32r, kind="ExternalInput")
    cg9_d = nc.dram_tensor("cg9", [126, NG], dt.float32, kind="ExternalInput")
    wc9_d = nc.dram_tensor("wc9", [126, NG, C2], dt.bfloat16, kind="ExternalInput")
    a2c2_d = nc.dram_tensor("a2c2", [C_, 2, 2], dt.float32, kind="ExternalInput")
    out_d = nc.dram_tensor("out", [C2, S], dt.float32, kind="ExternalOutput")
    FW = 80 * HP  # 6560: flat window length per shifted copy
    hp9d = nc.dram_tensor("hp9d", [9, C_, FW], dt.bfloat16)

    f32r = dt.float32r

    with tile.TileContext(nc) as tc:
        with tc.tile_pool(name="singles", bufs=1) as singles, \
             tc.tile_pool(name="strips", bufs=1) as strips, \
             tc.tile_pool(name="work", bufs=3) as work, \
             tc.tile_pool(name="obp", bufs=1) as obp, \
             tc.tile_pool(name="psz", bufs=2, space="PSUM") as psz, \
             tc.tile_pool(name="psd", bufs=1, space="PSUM") as psd, \
             tc.tile_pool(name="psr", bufs=1, space="PSUM") as psr, \
             tc.tile_pool(name="psh", bufs=2, space="PSUM") as psh, \
             tc.tile_pool(name="pso", bufs=1, space="PSUM") as pso:

            # ---- resident tiles + constant loads ----
            x2 = [singles.tile([C_, S], dt.float32r, tag=f"x{k}", name=f"x{k}") for k in range(2)]
            for k in range(2):
                nc.sync.dma_start(out=x2[k][:], in_=xs_d[k * C_:(k + 1) * C_, :])
            w1t = [singles.tile([C_, C_], dt.float32r, tag=f"w1t{k}", name=f"w1t{k}") for k in range(2)]
            for k in range(2):
                nc.sync.dma_start(out=w1t[k][:], in_=w1t_d[k * C_:(k + 1) * C_, :])
            a1c1 = singles.tile([C_, 2], dt.float32, tag="a1c1", name="a1c1")
            nc.sync.dma_start(out=a1c1[:], in_=a1c1_d[:])
            zb = singles.tile([126, NG, 126], dt.bfloat16, tag="zb", name="zb")
            nc.sync.dma_start(out=zb[:], in_=zb_d[:])
            ones_c = singles.tile([126, 2, GC], dt.bfloat16, tag="ones_c", name="ones_c")
            nc.sync.dma_start(out=ones_c[:], in_=ones_d[:])
            negrep = singles.tile([GC, 2, 126], dt.float32r, tag="negrep", name="negrep")
            nc.sync.dma_start(out=negrep[:], in_=negrep_d[:])
            cg9 = singles.tile([126, NG], dt.float32, tag="cg9", name="cg9")
            nc.sync.dma_start(out=cg9[:], in_=cg9_d[:])
            wc9 = singles.tile([126, NG, C2], dt.bfloat16, tag="wc9", name="wc9")
            nc.sync.dma_start(out=wc9[:], in_=wc9_d[:])
            a2c2 = singles.tile([C_, 2, 2], dt.float32, tag="a2c2", name="a2c2")
            nc.sync.dma_start(out=a2c2[:], in_=a2c2_d[:])

            hpfl = singles.tile([C_, HP * HP + 2], dt.bfloat16, tag="hp", name="hp")
            nc.vector.memset(hpfl[:], 0.0)
            hp = hpfl[:, 0:HP * HP].rearrange("p (a b) -> p a b", a=HP)

            # ---- phase A: cv1 into padded frame ----
            for ch in range(NCH):
                y0 = ch * CH_ROWS
                ph = psh.tile([C_, CS], dt.float32, tag="ph", name="ph")
                for k in range(2):
                    nc.tensor.matmul(
                        out=ph[:],
                        lhsT=w1t[k][:],
                        rhs=x2[k][:, y0 * W:(y0 + CH_ROWS) * W],
                        start=(k == 0), stop=(k == 1))
                yb = work.tile([C_, CS], dt.bfloat16, tag="yb", name="yb")
                nc.scalar.activation(out=yb[:], in_=ph[:], func=AF.Identity,
                                     scale=a1c1[:, 0:1], bias=a1c1[:, 1:2])
                sg = work.tile([C_, CS], dt.bfloat16, tag="sg", name="sg")
                nc.scalar.activation(out=sg[:], in_=ph[:], func=AF.Sigmoid,
                                     scale=a1c1[:, 0:1], bias=a1c1[:, 1:2])
                nc.vector.tensor_mul(hp[:, 1 + y0:1 + y0 + CH_ROWS, 1:1 + W],
                                     yb[:], sg[:])

            # ---- replicate hp into 9 shifted DRAM copies (full width) ----
            hpf = hpfl[:]
            for i in range(9):
                dy, dx = i // 3, i % 3
                d = dy * HP + dx
                nc.sync.dma_start(out=hp9d[i], in_=hpf[:, d:d + FW])

            # ---- phases B-E per super-chunk / group / chunk ----
            for sc in range(NSC):
                ys = sc * SC_ROWS
                hp9 = []
                for g in range(NG):
                    c0, ncg, P = _grp(g)
                    st = strips.tile([126, SC_ROWS, HP], dt.bfloat16, tag=f"hp9_{g}", name=f"hp9_{g}")
                    hp9.append(st)
                    srcap = bass.AP(
                        tensor=hp9d[0].tensor, offset=c0 * FW + ys * HP,
                        ap=[[C_ * FW, 9], [FW, ncg], [1, SC_ROWS * HP]])
                    nc.sync.dma_start(out=st[0:P, :, :], in_=srcap)

                ob2 = [obp.tile([C_, SC_ROWS * W], dt.float32, tag=f"ob2_{h}", name=f"ob2_{h}")
                       for h in range(2)]
                for sub in range(SC_ROWS // CH_ROWS):
                    ch = sc * (SC_ROWS // CH_ROWS) + sub
                    po = [pso.tile([C_, CS], dt.float32, tag=f"po{h}", name=f"po{h}")
                          for h in range(2)]
                    for g in range(NG):
                        c0, ncg, P = _grp(g)
                        v = 0 if ncg == GC else 1
                        rhs = hp9[g][0:P, sub * CH_ROWS:(sub + 1) * CH_ROWS, 0:W]
                        pz = psz.tile([126, CS], dt.float32, tag="pz", name="pz")
                        nc.tensor.matmul(out=pz[0:P, :], lhsT=zb[0:P, g, :][:, 0:P],
                                         rhs=rhs, start=True, stop=True)
                        e9 = work.tile([126, CS], dt.bfloat16, tag="e9", name="e9")
                        nc.scalar.activation(out=e9[0:P, :], in_=pz[0:P, :],
                                             func=AF.Exp, bias=cg9[0:P, g:g + 1])
                        pd = psd.tile([GC, CS], dt.float32, tag="pd", name="pd")
                        nc.tensor.matmul(out=pd[0:ncg, :],
                                         lhsT=ones_c[0:P, v, :][:, 0:ncg],
                                         rhs=e9[0:P, :], start=True, stop=True)
                        lnd = work.tile([GC, CS], dt.float32r, tag="lnd", name="lnd")
                        nc.scalar.activation(out=lnd[0:ncg, :], in_=pd[0:ncg, :],
                                             func=AF.Ln)
                        pr = psr.tile([126, CS], dt.float32, tag="pr", name="pr")
                        nc.tensor.matmul(out=pr[0:P, :],
                                         lhsT=negrep[0:ncg, v, :][:, 0:P],
                                         rhs=lnd[0:ncg, :],
                                         start=True, stop=True)
                        r9 = work.tile([126, CS], dt.bfloat16, tag="r9", name="r9")
                        nc.scalar.activation(out=r9[0:P, :], in_=pr[0:P, :],
                                             func=AF.Exp)
                        q1 = work.tile([126, CS], dt.bfloat16, tag="q1", name="q1")
                        nc.vector.tensor_mul(q1[0:P, :], rhs, e9[0:P, :])
                        q2 = work.tile([126, CS], dt.bfloat16, tag="q2", name="q2")
                        nc.vector.tensor_mul(q2[0:P, :], q1[0:P, :], r9[0:P, :])
                        for h in range(2):
                            nc.tensor.matmul(
                                out=po[h][:],
                                lhsT=wc9[0:P, g, h * C_:(h + 1) * C_],
                                rhs=q2[0:P, :],
                                start=(g == 0), stop=(g == NG - 1))
                    for h in range(2):
                        t = work.tile([C_, CS], dt.float32, tag=f"t{h}", name=f"t{h}")
                        nc.scalar.activation(out=t[:], in_=po[h][:], func=AF.Relu,
                                             scale=a2c2[:, h, 0:1],
                                             bias=a2c2[:, h, 1:2])
                        nc.vector.tensor_add(
                            ob2[h][:, sub * CS:(sub + 1) * CS], t[:],
                            x2[h][:, ch * CS:(ch + 1) * CS].bitcast(dt.float32))
                for h in range(2):
                    nc.sync.dma_start(
                        out=out_d[h * C_:(h + 1) * C_,
                                  sc * (SC_ROWS * W):(sc + 1) * (SC_ROWS * W)],
                        in_=ob2[h][:])

    _split_excess_waits(nc)
    return nc


def _split_excess_waits(nc):
    """This walrus build rejects >1 sync-wait on TPB_CTRL instructions and
    >2 elsewhere; redistribute onto same-engine wait-nops inserted before."""
    import concourse.mybir as mybir
    cnt = [0]
    for bb in nc.main_func.blocks:
        new_list = []
        changed = False
        for ins in bb.instructions:
            si = ins.sync_info
            lim = 1
            if si is not None and si.on_wait is not None and len(si.on_wait) > lim:
                waits = list(si.on_wait)
                head, tail = waits[:-lim], waits[-lim:]
                for w in head:
                    nop = mybir.InstNoOp(name=f"waitsplit-{cnt[0]}", ins=[], outs=[])
                    cnt[0] += 1
                    nop.engine = ins.engine
                    nop.sync_info = mybir.SyncInfo(on_wait=[w], on_update=[])
                    nop.bass_nofuse = True
                    try:
                        nc.register_instruction(nop)
                    except Exception:
                        pass
                    new_list.append(nop)
                ins.sync_info = mybir.SyncInfo(
                    on_wait=tail, on_update=list(si.on_update or []))
                changed = True
            new_list.append(ins)
        if changed:
            bb.instructions[:] = new_list


def _get_program():
    global _PROGRAM
    if _PROGRAM is None:
        _PROGRAM = _build_program()
    return _PROGRAM


def kernel(**inputs):
    from concourse.bass_utils import run_bass_kernel_spmd

    x = np.asarray(inputs['x'], dtype=np.float32)
    cst = _fold_constants(**{k: np.asarray(v, dtype=np.float32)
                             for k, v in inputs.items() if k != 'x'})
    nc = _get_program()
    base = {
        'w1t': cst['w1t'], 'a1c1': cst['a1c1'], 'zb': cst['zb'],
        'ones_c': cst['ones_c'], 'negrep': cst['negrep'], 'cg9': cst['cg9'],
        'wc9': cst['wc9'], 'a2c2': cst['a2c2'],
    }
    in_maps = [dict(base, xs=np.ascontiguousarray(x[b].reshape(C1, S)))
               for b in range(B)]
    res = run_bass_kernel_spmd(nc, in_maps, list(range(B)))
    out = np.stack([res.results[b]['out'].reshape(C2, H, W) for b in range(B)])
    return out.astype(np.float32)

